# revision 1
# baseline (speedup 1.0000x reference)
"""Trainium2 Bass kernel for the ChaosModulator recurrence.

Math (per (b,c) sequence, t = 0..4095):
    sigma_t = 3.5*z*(1-z) + 0.5*x_t
    z'      = 0.5*z + 0.25*(1 + tanh(sigma_t))        (clip is a no-op: z' in (0,1))
    u_t     = 0.5*x_t + 0.5*(2*z' - 1)

Reformulated with w = 2z-1 and state s_t = w_t + h_t (so w_{t+1} = 0.5*s_t):
    e_t = x_t - (7/16)*s_{t-1}^2
    h_t = tanh(0.5*e_t + 0.875)
    s_t = 0.5*s_{t-1} + h_t
    u_t = 0.25*s_t + 0.5*x_t

The map contracts with factor ~0.5/step, so each 128-step time block can be
computed independently after a 32-step warmup from an arbitrary state
(validated: fp64-exact at W=32).  This turns the serial t-loop into
32 independent chains per sequence -> wide [128 x 128] per-step ops.

Per step: 1 ACT op (tanh) + 2 DVE ops (stt s-update, fused CHAOS_E e-update).
States for output steps are written into dead X-tile slots; u is produced by
one bulk fused CHAOS_U op per batch.

Sharding: batch dim b (32) split 4-per-core across 8 cores; per core
2048 sequences x 4096 steps.
"""

import numpy as np

import concourse.bacc as bacc
import concourse.dve_ops as dve_ops
import concourse.mybir as mybir
from concourse.bass_utils import run_bass_kernel_spmd
from concourse.dve_spec import C0, C1, Spec, Src0, Src1, _has_src1, lower, sq
from concourse.dve_uop import DveOpSpec
from concourse.tile import TileContext

F32 = mybir.dt.float32
P = 128             # SBUF partitions
G = 16              # sequence groups per core (2048 = G*P)
T = 4096
B = 128             # output steps per block
W = 32              # warmup steps per block
L = B + W           # chain length
NBLK = T // B       # 64 blocks
BLK_PER_BATCH = 8
NBATCH = NBLK // BLK_PER_BATCH   # 8
NCOLS = BLK_PER_BATCH * G        # 128 chain-columns per batch
NSEQ = P * G        # 2048 sequences per core
NCORES = 8
XBUFS = 2           # batches in flight (SBUF: XBUFS * 80KB/partition)

_MULT = mybir.AluOpType.mult
_ADD = mybir.AluOpType.add


def _register_custom_ops():
    """Register the two fused DVE ops (idempotent)."""
    if "CHAOS_E" in dve_ops._SUB_OPCODE_FOR_NAME:
        by = {op.name: op for op in dve_ops.OPS}
        return by["CHAOS_E"], by["CHAOS_U"]

    spec_e = Spec(
        body=Src1 - C0 * sq(Src0),
        reference=lambda in0, in1, s0: in1 - s0 * in0 * in0,
    )
    spec_u = Spec(
        body=C0 * Src0 + C1 * Src1,
        reference=lambda in0, in1, s0, s1: s0 * in0 + s1 * in1,
    )
    ops = []
    for name, spec in (("CHAOS_E", spec_e), ("CHAOS_U", spec_u)):
        op = dve_ops.DveOp(name, spec, subdim=False, uops_sha={})
        dve_ops.OPS.append(op)
        dve_ops.CUSTOM_DVE_SPECS[name] = spec
        dve_ops._SUB_OPCODE_FOR_NAME[name] = (
            dve_ops._CUSTOM_DVE_ROW_BASE + len(dve_ops.OPS) - 1
        )
        # pin the uops sha self-consistently
        for ver in ("v3", "v4"):
            try:
                s = DveOpSpec(
                    name=name,
                    opcode=dve_ops.get_dve_sub_opcode(name),
                    uops=lower(spec, ver=ver),
                    rd1_en=_has_src1(spec),
                )
                op.uops_sha[ver] = s.sha(ver)
            except Exception:
                pass
        ops.append(op)
    return ops


def _build_nc():
    CHAOS_E, CHAOS_U = _register_custom_ops()
    nc = bacc.Bacc("TRN2", target_bir_lowering=False, debug=False)

    x = nc.dram_tensor("x", [NSEQ, T], F32, kind="ExternalInput")
    z0 = nc.dram_tensor("z0", [NSEQ], F32, kind="ExternalInput")
    u = nc.dram_tensor("u", [NSEQ, T], F32, kind="ExternalOutput")

    xr = x[:, :].rearrange("(g p) t -> p g t", p=P)    # [128, 16, 4096]
    ur = u[:, :].rearrange("(g p) t -> p g t", p=P)
    z0r = z0[:].rearrange("(g p) -> p g", p=P)         # [128, 16]

    with TileContext(nc) as tc:
        with (
            tc.tile_pool(name="xp", bufs=XBUFS) as xp,
            tc.tile_pool(name="sp", bufs=XBUFS) as sp,
            tc.tile_pool(name="cp", bufs=1) as cp,
        ):
            z0_t = cp.tile([P, G], F32)
            nc.sync.dma_start(out=z0_t[:, :], in_=z0r)
            # s_init = 4*z0 - 2  (so that w_0 = 0.5*s_init = 2*z0 - 1)
            s_init = cp.tile([P, G], F32)
            nc.vector.tensor_scalar(
                out=s_init[:, :], in0=z0_t[:, :],
                scalar1=4.0, scalar2=-2.0, op0=_MULT, op1=_ADD,
            )
            # per-partition bias for tanh(0.5*e + 0.875)
            bias_t = cp.tile([P, 1], F32)
            nc.vector.memset(bias_t[:, :], 0.875)

            for bt in range(NBATCH):
                # X tile: [c][k] layout, c = nl*G + g, k = chain step.
                # Slots hold raw x; slots 0..B-1 are progressively reused to
                # hold the state s_k (slot k-W) and finally u.
                Xt = xp.tile([P, NCOLS * L], F32, name=f"X{bt}", tag="X")
                Xv = Xt.rearrange("p (c k) -> p c k", k=L)

                for nl in range(BLK_PER_BATCH):
                    n = bt * BLK_PER_BATCH + nl
                    cs = nl * G
                    if n == 0:
                        # block 0 starts at t=-W: pad warmup with zeros
                        nc.vector.memset(Xv[:, cs:cs + G, 0:W], 0.0)
                        nc.sync.dma_start(
                            out=Xv[:, cs:cs + G, W:L], in_=xr[:, :, 0:B]
                        )
                    else:
                        t0 = n * B - W
                        nc.sync.dma_start(
                            out=Xv[:, cs:cs + G, :], in_=xr[:, :, t0:t0 + L]
                        )

                h_t = sp.tile([P, NCOLS], F32, name=f"h{bt}", tag="h")
                e_t = [
                    sp.tile([P, NCOLS], F32, name=f"e{bt}_{i}", tag=f"e{i}")
                    for i in range(2)
                ]
                s_t = [
                    sp.tile([P, NCOLS], F32, name=f"s{bt}_{i}", tag=f"s{i}")
                    for i in range(2)
                ]

                nc.vector.memset(s_t[0][:, :], 0.0)
                # e_0 = x_0 - (7/16)*0^2 = x_0
                nc.vector.tensor_copy(out=e_t[0][:, :], in_=Xv[:, :, 0])

                for k in range(L):
                    cur, nxt = k % 2, (k + 1) % 2
                    # h = tanh(0.5*e + 0.875)
                    nc.scalar.activation(
                        out=h_t[:, :], in_=e_t[cur][:, :],
                        func=mybir.ActivationFunctionType.Tanh,
                        bias=bias_t[:, :], scale=0.5,
                    )
                    # s' = 0.5*s + h ; for k>=W write into dead X slot k-W
                    s_prev = s_t[cur][:, :] if k <= W else Xv[:, :, k - 1 - W]
                    s_out = s_t[nxt][:, :] if k < W else Xv[:, :, k - W]
                    nc.vector.scalar_tensor_tensor(
                        out=s_out, in0=s_prev, scalar=0.5,
                        in1=h_t[:, :], op0=_MULT, op1=_ADD,
                    )
                    if bt == 0 and k == W - 1:
                        # block 0: replace warmup state with the true z0 state
                        nc.vector.tensor_copy(
                            out=s_t[nxt][:, 0:G], in_=s_init[:, :]
                        )
                        s_out = s_t[nxt][:, :]
                    if k < L - 1:
                        # e' = x_{k+1} - (7/16)*s'^2
                        nc.vector._custom_dve(
                            CHAOS_E, out=e_t[nxt][:, :], in0=s_out,
                            in1=Xv[:, :, k + 1], s0=0.4375,
                        )
                    # chunked bulk u = 0.25*s_j + 0.5*x_j for j in [k-16, k):
                    # s_j sits in slot j-W (last read by step j+1 <= k, done),
                    # x_j in slot j (destroyed at step j+W >= k+16, alive).
                    if k >= W + 16 and (k - W) % 16 == 0:
                        lo = k - W - 16
                        nc.vector._custom_dve(
                            CHAOS_U,
                            out=Xv[:, :, lo:lo + 16], in0=Xv[:, :, lo:lo + 16],
                            in1=Xv[:, :, k - 16:k], s0=0.25, s1=0.5,
                        )

                # final u chunk: j in [L-16, L)
                nc.vector._custom_dve(
                    CHAOS_U,
                    out=Xv[:, :, B - 16:B], in0=Xv[:, :, B - 16:B],
                    in1=Xv[:, :, L - 16:L], s0=0.25, s1=0.5,
                )

                for nl in range(BLK_PER_BATCH):
                    n = bt * BLK_PER_BATCH + nl
                    cs = nl * G
                    nc.sync.dma_start(
                        out=ur[:, :, n * B:(n + 1) * B],
                        in_=Xv[:, cs:cs + G, 0:B],
                    )

    nc.compile()
    return nc


_NC = None


def _get_nc():
    global _NC
    if _NC is None:
        _NC = _build_nc()
    return _NC


def kernel(x: np.ndarray, z0: np.ndarray) -> np.ndarray:
    x = np.ascontiguousarray(x, dtype=np.float32)      # (32, 512, 4096)
    z0 = np.ascontiguousarray(z0, dtype=np.float32)    # (32, 512)
    nc = _get_nc()

    in_maps = []
    for i in range(NCORES):
        xs = np.ascontiguousarray(x[4 * i:4 * (i + 1)].reshape(NSEQ, T))
        zs = np.ascontiguousarray(z0[4 * i:4 * (i + 1)].reshape(NSEQ))
        in_maps.append({"x": xs, "z0": zs})

    res = run_bass_kernel_spmd(nc, in_maps, core_ids=list(range(NCORES)))
    out = np.empty((32, 512, T), np.float32)
    for i in range(NCORES):
        out[4 * i:4 * (i + 1)] = res.results[i]["u"].reshape(4, 512, T)
    return out



# revision 6
# speedup vs baseline: 1.0876x; 1.0876x over previous
"""Trainium2 Bass kernel for the ChaosModulator recurrence (v3).

Math (per (b,c) sequence, t = 0..4095), with v = 0.25*(2z-1 + tanh-part):
    e_t = x_t - 7*v_{t-1}^2
    h_t = tanh(0.5*e_t + 0.875)
    v_t = 0.5*v_{t-1} + 0.25*h_t          (v = 0.25*s of the s-form)
    u_t = 0.5*x_t + v_t

The map contracts ~0.5/step, so each 16-step output span can be computed by
an independent chain warmed up W=8 steps from v=0 (validated: rel-l2 2.5e-3
incl. bf16 storage, vs 2e-2 budget).  4096 steps -> 256 sub-chains per
sequence, processed as 1024-wide SIMD columns (16 groups x 64 sub-chains)
over 4 time-batches of 1024 steps.

Layout per core (2048 seqs = 128 partitions x 16 groups):
  Xb[p, g, 1040] bf16  row-major x (gpsimd cast-DMA f32->bf16, 1 DMA/batch)
  U [p, g, 1024] bf16  v-state at final u position; bulk op turns it into u;
                       gpsimd cast-DMA bf16->f32 writes it out contiguously.
Chain ops are [128 x 1024] custom DVE ops (LIN = c0*v + c1*h, CHAOS_E =
x - 7*v^2) + one ACT tanh per round; 24 rounds/batch, 2 batches interleaved.
"""

import numpy as np

import concourse.bacc as bacc
import concourse.dve_ops as dve_ops
import concourse.mybir as mybir
from concourse.bass_utils import run_bass_kernel_spmd
from concourse.dve_spec import C0, C1, Spec, Src0, Src1, _has_src1, lower, sq
from concourse.dve_uop import DveOpSpec
from concourse.tile import TileContext

F32 = mybir.dt.float32
BF16 = mybir.dt.bfloat16

P = 128             # SBUF partitions
G = 16              # sequence groups per core (2048 = G*P)
T = 4096
S = 16              # sub-chain output steps
W = 8               # warmup steps
CH = 1024           # time-steps per batch
NBATCH = T // CH    # 4
NSUB = CH // S      # 64 sub-chains per (g, batch)
C = G * NSUB        # 1024 chain columns per batch
XW = CH + S         # x tile width (c-viewable: 65*16); only CH+W loaded
NSEQ = P * G
NCORES = 8

_MULT = mybir.AluOpType.mult
_ADD = mybir.AluOpType.add


def _register_custom_ops():
    """Register the two fused DVE ops (idempotent)."""
    if "CHAOS_E" in dve_ops._SUB_OPCODE_FOR_NAME:
        by = {op.name: op for op in dve_ops.OPS}
        return by["CHAOS_E"], by["CHAOS_U"]

    spec_e = Spec(
        body=Src1 - C0 * sq(Src0),
        reference=lambda in0, in1, s0: in1 - s0 * in0 * in0,
    )
    spec_u = Spec(
        body=C0 * Src0 + C1 * Src1,
        reference=lambda in0, in1, s0, s1: s0 * in0 + s1 * in1,
    )
    ops = []
    for name, spec in (("CHAOS_E", spec_e), ("CHAOS_U", spec_u)):
        op = dve_ops.DveOp(name, spec, subdim=False, uops_sha={})
        dve_ops.OPS.append(op)
        dve_ops.CUSTOM_DVE_SPECS[name] = spec
        dve_ops._SUB_OPCODE_FOR_NAME[name] = (
            dve_ops._CUSTOM_DVE_ROW_BASE + len(dve_ops.OPS) - 1
        )
        for ver in ("v3", "v4"):
            try:
                s = DveOpSpec(
                    name=name,
                    opcode=dve_ops.get_dve_sub_opcode(name),
                    uops=lower(spec, ver=ver),
                    rd1_en=_has_src1(spec),
                )
                op.uops_sha[ver] = s.sha(ver)
            except Exception:
                pass
        ops.append(op)
    return ops


def _build_nc():
    CHAOS_E, CHAOS_U = _register_custom_ops()
    nc = bacc.Bacc("TRN2", target_bir_lowering=False, debug=False)

    x = nc.dram_tensor("x", [NSEQ, T], F32, kind="ExternalInput")
    z0 = nc.dram_tensor("z0", [NSEQ], F32, kind="ExternalInput")
    u = nc.dram_tensor("u", [NSEQ, T], F32, kind="ExternalOutput")

    xr = x[:, :].rearrange("(g p) t -> p g t", p=P)    # [128, 16, 4096]
    ur = u[:, :].rearrange("(g p) t -> p g t", p=P)
    z0r = z0[:].rearrange("(g p) -> p g", p=P)         # [128, 16]

    with TileContext(nc) as tc:
        with (
            tc.tile_pool(name="xp", bufs=2) as xp,
            tc.tile_pool(name="up", bufs=2) as up,
            tc.tile_pool(name="sp", bufs=2) as sp,
            tc.tile_pool(name="cp", bufs=1) as cp,
        ):
            # constants / init
            bias_t = cp.tile([P, 1], F32, name="bias")
            nc.vector.memset(bias_t[:, :], 0.875)
            z0_t = cp.tile([P, G], F32, name="z0t")
            nc.sync.dma_start(out=z0_t[:, :], in_=z0r)
            # v_init = 0.25*(4*z0-2) = z0 - 0.5
            z0p = cp.tile([P, G], F32, name="z0p")
            nc.scalar.activation(
                out=z0p[:, :], in_=z0_t[:, :],
                func=mybir.ActivationFunctionType.Copy, bias=-0.5)

            tiles = {}
            for bt in range(NBATCH):
                Xb = xp.tile([P, G, XW], BF16, name=f"X{bt}", tag="X")
                if bt == 0:
                    nc.vector.memset(Xb[:, :, 0:W], 0.0)
                    nc.gpsimd.dma_start(
                        out=Xb[:, :, W:W + CH], in_=xr[:, :, 0:CH])
                else:
                    t0 = bt * CH - W
                    nc.gpsimd.dma_start(
                        out=Xb[:, :, 0:W + CH], in_=xr[:, :, t0:t0 + W + CH])
                Ub = up.tile([P, G, CH], BF16, name=f"U{bt}", tag="U")
                h2 = [sp.tile([P, G, NSUB], F32, name=f"h{bt}_{i}", tag=f"h{i}")
                      for i in range(2)]
                e2 = [sp.tile([P, G, NSUB], F32, name=f"e{bt}_{i}", tag=f"e{i}")
                      for i in range(2)]
                vw = [sp.tile([P, G, NSUB], F32, name=f"v{bt}_{i}", tag=f"v{i}")
                      for i in range(2)]
                tiles[bt] = (Xb, Ub, h2, e2, vw)

            def round_ops(bt, k):
                Xb, Ub, h2, e2, vw = tiles[bt]
                Xv = Xb.rearrange("p g (c s) -> p g c s", s=S)
                Uv = Ub.rearrange("p g (c s) -> p g c s", s=S)
                h = h2[k % 2][:, :, :]
                # h_k = tanh(0.5*e_k + 0.875); e_0 = x_0 read straight from X
                src = Xv[:, :, 0:NSUB, 0] if k == 0 else e2[k % 2][:, :, :]
                nc.scalar.activation(
                    out=h, in_=src,
                    func=mybir.ActivationFunctionType.Tanh,
                    bias=bias_t[:, :], scale=0.5)
                # v_k = 0.5*v_{k-1} + 0.25*h_k   (v_{-1} = 0)
                if k == 0:
                    v_prev = h
                    s0, s1 = 0.125, 0.125
                else:
                    v_prev = (vw[(k - 1) % 2][:, :, :] if k <= W
                              else Uv[:, :, :, k - 1 - W])
                    s0, s1 = 0.5, 0.25
                v_out = vw[k % 2][:, :, :] if k < W else Uv[:, :, :, k - W]
                nc.vector._custom_dve(
                    CHAOS_U, out=v_out, in0=v_prev, in1=h, s0=s0, s1=s1)
                if bt == 0 and k == W - 1:
                    # replace warmup state of the t=0 sub-chain with true z0
                    nc.vector.tensor_copy(
                        out=vw[k % 2][:, :, 0], in_=z0p[:, :])
                    v_out = vw[k % 2][:, :, :]
                # e_{k+1} = x_{k+1} - 7*v_k^2
                if k < W + S - 1:
                    kk = k + 1
                    xin = (Xv[:, :, 0:NSUB, kk] if kk < S
                           else Xv[:, :, 1:NSUB + 1, kk - S])
                    nc.vector._custom_dve(
                        CHAOS_E, out=e2[(k + 1) % 2][:, :, :], in0=v_out,
                        in1=xin, s0=7.0)

            def tail_ops(bt):
                Xb, Ub, h2, e2, vw = tiles[bt]
                # u = 1.0*v + 0.5*x  (in place over U)
                nc.vector._custom_dve(
                    CHAOS_U, out=Ub[:, :, :], in0=Ub[:, :, :],
                    in1=Xb[:, :, W:W + CH], s0=1.0, s1=0.5)
                nc.gpsimd.dma_start(
                    out=ur[:, :, bt * CH:(bt + 1) * CH], in_=Ub[:, :, :])

            for pair in range(NBATCH // 2):
                b0, b1 = 2 * pair, 2 * pair + 1
                for k in range(W + S):
                    round_ops(b0, k)
                    round_ops(b1, k)
                tail_ops(b0)
                tail_ops(b1)

    nc.compile()
    return nc


_NC = None


def _get_nc():
    global _NC
    if _NC is None:
        _NC = _build_nc()
    return _NC


def kernel(x: np.ndarray, z0: np.ndarray) -> np.ndarray:
    x = np.ascontiguousarray(x, dtype=np.float32)      # (32, 512, 4096)
    z0 = np.ascontiguousarray(z0, dtype=np.float32)    # (32, 512)
    nc = _get_nc()

    in_maps = []
    for i in range(NCORES):
        xs = np.ascontiguousarray(x[4 * i:4 * (i + 1)].reshape(NSEQ, T))
        zs = np.ascontiguousarray(z0[4 * i:4 * (i + 1)].reshape(NSEQ))
        in_maps.append({"x": xs, "z0": zs})

    res = run_bass_kernel_spmd(nc, in_maps, core_ids=list(range(NCORES)))
    out = np.empty((32, 512, T), np.float32)
    for i in range(NCORES):
        out[4 * i:4 * (i + 1)] = res.results[i]["u"].reshape(4, 512, T)
    return out
